# revision 8
# baseline (speedup 1.0000x reference)
"""Trainium2 Bass kernel for nn_InpaintContextAttentionUnit.

Per-sample computation (B=8 samples -> 1 per NeuronCore):
  fm [512,512,16] -> avgpool(64x2) -> pooled [8,256,16]
  -> two masked 3x3 convs (middle row / middle col of kernel zeroed) + bias + relu
  -> bilinear upsample back to [512,512,16] (separable; half-pixel centers, edge clamp)
  -> out [512,512,48] = concat(fm, fm - row_up, fm - col_up)

Design v3 — fully pipelined loads/compute/stores (v2 ran loads, then a
40us DMA-dead conv/upsample phase, then stores; v3 streams stores from
~35us):
  - per-output-tile H-upsample slicing: output rows 128t..128t+127 only
    read pooled conv rows 2t-1..2t+2, so the hup matmul contracts over a
    4-partition slice instead of all 8; output tile t then only depends
    on input tiles t-1..t+1
  - schedule: load t0,t1 -> pool -> conv rows 0-2 -> W-up rows 0-2 ->
    out tile 0 streams while t2,t3 load; conv 3-4 / W-up / out tile 1
    after t2; conv 5-7 / W-up / out tiles 2,3 after t3. HBM never idles.
  - W-upsample STT ops and memsets run on GpSimd (otherwise idle); DVE
    is the steady-state-limiting compute engine (pass-B subtracts)
  - fm is loaded once (8x 2 MiB), cast to a persistent bf16 copy used by
    pooling, the pass-through channel, and the subtracts (bf16 rel err
    ~2^-9, well under the 2e-2 gate)
  - pooling via PE matmul with a [128,2] block-mean matrix; conv via
    accumulating [48c,16f]x[48c,512] matmuls over a 3-shift stacked
    input (dwp taps on partitions); per-row-range DRAM bounces reorder
    conv output to (n-on-partition, f, w) for the W-up
All constant matrices are precomputed on host and passed as extra inputs.
"""

import numpy as np
import ml_dtypes

H, W, C, F = 512, 512, 16, 16
NPOOL = 8
WP = W // 2  # 256
CH_OUT = 3 * C  # 48

_cache = {}


def _host_consts(kernel, bias):
    """Build host-side constant matrices (bf16 for the PE-side constants)."""
    bf = ml_dtypes.bfloat16
    # pooling weights: [128, 2], 1/128 (exact in bf16) where row block matches
    poolw = np.zeros((128, 2), np.float32)
    poolw[:64, 0] = 1.0 / 128.0
    poolw[64:, 1] = 1.0 / 128.0
    # H-upsample matrix: hup[n, y] = weight of pooled row n for output row y,
    # scaled by 0.75 (the W-upsample major tap; k/64*0.75 = 3k/256 exact in bf16)
    hup = np.zeros((NPOOL, H), np.float32)
    scale = H // NPOOL
    for y in range(H):
        yf = (y + 0.5) / scale - 0.5
        i0 = int(np.floor(yf))
        w = yf - i0
        hup[min(max(i0, 0), NPOOL - 1), y] += 1.0 - w
        hup[min(max(i0 + 1, 0), NPOOL - 1), y] += w
    hup *= 0.75
    hup2 = np.zeros((40, H), np.float32)
    hup2[0:8] = hup
    hup2[32:40] = hup  # col-branch copy at base partition 32; rows 8-15 stay zero
    # conv taps, stacked over dwp on 48 partitions (partition 16g+c holds the
    # dwp=g-1 shifted pooled copy). One matmul slot per (branch, dn):
    #   branch 0 (row conv): kernel[dn+1, dwp+1]; slots 0,1 for dn=-1,+1
    #   branch 1 (col conv): kernel[dwp+1, dn+1], dwp=0 block zero; slots 2-4
    kt = np.zeros((48, 5 * 16), np.float32)  # [(g,c), slot*16+f]
    for g in range(3):
        for s, dn in enumerate((-1, 1)):
            kt[16 * g:16 * (g + 1), s * 16:(s + 1) * 16] = kernel[dn + 1, g]
        for s, dn in enumerate((-1, 0, 1)):
            if g != 1:
                kt[16 * g:16 * (g + 1), (2 + s) * 16:(3 + s) * 16] = \
                    kernel[g, dn + 1]
    bias2 = np.ascontiguousarray(bias.reshape(16, 1)).astype(np.float32)
    return (poolw.astype(bf), hup2.astype(bf), kt.astype(bf), bias2, None, None)


def _build_program(compile=True):
    import concourse.bass as bass
    import concourse.bacc as bacc
    import concourse.mybir as mybir
    import concourse.tile as tile

    dt = mybir.dt.float32
    db = mybir.dt.bfloat16
    nc = bacc.Bacc()

    fm_d = nc.declare_dram_parameter("feature_map", [H, W, C], dt, isOutput=False)
    poolw_d = nc.declare_dram_parameter("poolw", [128, 2], db, isOutput=False)
    hup_d = nc.declare_dram_parameter("hup", [40, H], db, isOutput=False)
    ktaps_d = nc.declare_dram_parameter("ktaps", [48, 80], db, isOutput=False)
    bias_d = nc.declare_dram_parameter("bias2", [16, 1], dt, isOutput=False)
    out_d = nc.declare_dram_parameter("out", [H, W, CH_OUT], dt, isOutput=True)

    # matmul slots per branch: (slot, dn)
    slots_by_branch = [[(0, -1), (1, 1)], [(2, -1), (3, 0), (4, 1)]]

    with tile.TileContext(nc) as tc:
        with (
            tc.tile_pool(name="consts", bufs=1) as cpool,
            tc.tile_pool(name="persist", bufs=1) as ppool,
            tc.tile_pool(name="work", bufs=1) as wpool,
            tc.tile_pool(name="dram", bufs=1, space="DRAM") as dpool,
            tc.tile_pool(name="psall", bufs=1, space="PSUM") as psall,
        ):
            poolw_t = cpool.tile([128, 2], db)
            hup_t = cpool.tile([40, H], db)
            ktaps_t = cpool.tile([48, 80], db)
            bias_t = cpool.tile([16, 1], dt)

            # persistent bf16 fm copy: [128, (4 t, 512 x, 16 c)]
            fmb_t = ppool.tile([128, 4 * W * C], db)
            # rw [40, (16 f, 512 x)] bf16: partitions 0-7 row-branch, 32-39 col
            rw_t = ppool.tile([40, 16 * W], db)

            tpad_t = ppool.tile([48, 10 * 258], db)
            t48 = tpad_t[:].rearrange("p (n w) -> p n w", w=258)
            conv_t = ppool.tile([16, 2 * NPOOL * WP], db)
            rop_t = ppool.tile([40, 16 * 258], db)
            rop3 = rop_t[:].rearrange("p (f w) -> p f w", w=258)
            rwv = rw_t[:].rearrange("p (f xp par) -> p f par xp", par=2, xp=WP)
            rwx = rw_t[:].rearrange("p (f x) -> p f x", x=W)
            fmb4 = fmb_t[:].rearrange("p (t x c) -> p t x c", t=4, c=16)

            ncw_dram = dpool.tile([NPOOL, 16 * 258], db)
            nd3 = ncw_dram[:].rearrange("n (c w) -> n c w", w=258)
            ncwd3 = ncw_dram[:].rearrange("n (c w) -> c n w", w=258)
            conv_dram = dpool.tile([16, 2 * NPOOL * WP], db)
            cd4 = conv_dram[:].rearrange("f (b n w) -> b n f w", b=2, n=NPOOL)
            zsrc = hup_d[8:16, 0:16]  # [8, 16] zeros

            # zero-fill the t48 halo rows (0 and 9) and rop (rows 8-31 are
            # read zero-weighted by the wup spans); GpSimd is otherwise idle
            nc.gpsimd.memset(tpad_t[:], 0.0)
            nc.gpsimd.memset(rop_t[:], 0.0)

            # pooling rhs view of fmb: (t, xp, par, c) — c-inner contiguous
            fmr = fmb_t[:].rearrange(
                "p (t xp par c) -> p t xp par c", t=4, par=2, c=16)

            def load_tile(t):
                fmfs = []
                for h in range(2):
                    fmf = wpool.tile([128, W * C // 2], dt, tag="fmf", bufs=2,
                                     name=f"fmf{t}{h}")
                    fmf3 = fmf[:].rearrange("p (x c) -> p x c", c=C)
                    nc.sync.dma_start(
                        out=fmf3,
                        in_=fm_d[128 * t:128 * (t + 1),
                                 256 * h:256 * (h + 1)])
                    fmfs.append(fmf)
                return fmfs

            def cast_tile(t, fmfs):
                half = W * C // 2
                nc.scalar.activation(
                    out=fmb_t[:, t * W * C:t * W * C + half],
                    in_=fmfs[0][:],
                    func=mybir.ActivationFunctionType.Copy)
                nc.vector.tensor_copy(
                    fmb_t[:, t * W * C + half:(t + 1) * W * C],
                    fmfs[1][:])

            def pool_tile(t):
                # H-pool (y->n) + W-pair add; (xp, c)-major psum in 1-bank
                # eighths, f32->bf16 CAST alternating DVE/ACT
                stage = wpool.tile([2, WP * 16], db, tag="stage", bufs=1,
                                   name=f"stage{t}")
                for e in range(8):
                    ps = psall.tile([2, 512], dt, tag="pool", bufs=2,
                                    name=f"psp{t}{e}")
                    for par in range(2):
                        nc.tensor.matmul(
                            ps[:], poolw_t[:],
                            fmr[:, t, 32 * e:32 * (e + 1), par, :],
                            start=(par == 0), stop=(par == 1),
                        )
                    dst = stage[:, 512 * e:512 * (e + 1)]
                    if e < 2:
                        nc.vector.tensor_copy(dst, ps[:])
                    else:
                        nc.scalar.activation(
                            out=dst, in_=ps[:],
                            func=mybir.ActivationFunctionType.Copy)
                # free-dim transpose (xp, c) -> (c, w) on DVE halves
                stageT = wpool.tile([2, WP * 16], db, tag="stageT", bufs=1,
                                    name=f"stageT{t}")
                st_cx = stage[:].rearrange("p (x c) -> p c x", c=16)
                stT3 = stageT[:].rearrange("p (c x) -> p c x", x=WP)
                nc.vector.tensor_copy(stT3[:, 0:8, :], st_cx[:, 0:8, :])
                nc.vector.tensor_copy(stT3[:, 8:16, :], st_cx[:, 8:16, :])
                # bounce to DRAM; read back 3 dwp-shifted copies with
                # c on partitions (n rows shifted +1 for the zero halo)
                nc.sync.dma_start(
                    out=nd3[2 * t:2 * t + 2, :, 1:257], in_=stT3)
                for g in range(3):
                    nc.sync.dma_start(
                        out=t48[16 * g:16 * (g + 1),
                                2 * t + 1:2 * t + 3, 1:257],
                        in_=ncwd3[:, 2 * t:2 * t + 2, g:g + 256])

            def conv_unit(b, n0, nn):
                # conv rows n0..n0+nn; dwp taps contracted via the
                # 48-partition stack, one accumulating matmul per dn
                ps = psall.tile([16, 2 * WP], dt, tag="conv", bufs=2,
                                name=f"psc{b}{n0}")
                slots = slots_by_branch[b]
                for k, (sl, dn) in enumerate(slots):
                    nc.tensor.matmul(
                        ps[:, 0:nn * WP],
                        ktaps_t[:, sl * 16:(sl + 1) * 16],
                        t48[:, n0 + dn + 1:n0 + dn + 1 + nn, 1:257],
                        start=(k == 0), stop=(k == len(slots) - 1),
                    )
                nc.scalar.activation(
                    out=conv_t[:, (b * NPOOL + n0) * WP:
                               (b * NPOOL + n0 + nn) * WP],
                    in_=ps[:, 0:nn * WP],
                    func=mybir.ActivationFunctionType.Relu,
                    bias=bias_t[:, 0:1],
                )

            def tail(b, nlo, nhi):
                # bounce conv rows [nlo,nhi) to [(b,n) parts, (f, wp)]
                nc.sync.dma_start(
                    out=conv_dram[:, (b * NPOOL + nlo) * WP:
                                  (b * NPOOL + nhi) * WP],
                    in_=conv_t[:, (b * NPOOL + nlo) * WP:
                               (b * NPOOL + nhi) * WP])
                nc.sync.dma_start(
                    out=rop3[32 * b + nlo:32 * b + nhi, :, 1:257],
                    in_=cd4[b][nlo:nhi])

            def wup_round(nhi):
                # W-upsample conv rows [0,nhi) of both branches into
                # (f, x)-major rw; 0.75 is folded into hup:
                #   rw[2k] = pad[k]/3 + pad[k+1]; rw[2k+1] = pad[k+2]/3 + pad[k+1]
                # One partition-spanning op pair (GpSimd base must be 0/32;
                # rows 8-31 compute junk from the zeroed rop, never read;
                # re-spanned lower rows recompute identical values).
                # Edge copies on GpSimd; the STTs must run on DVE
                # (TensorScalarPtr is not supported on the Pool engine).
                third = 1.0 / 3.0
                rows = slice(0, 32 + nhi)
                nc.gpsimd.tensor_copy(
                    rop3[rows, :, 0:1], rop3[rows, :, 1:2])
                nc.gpsimd.tensor_copy(
                    rop3[rows, :, 257:258], rop3[rows, :, 256:257])
                nc.vector.scalar_tensor_tensor(
                    out=rwv[rows, :, 0, :],
                    in0=rop3[rows, :, 0:256],
                    scalar=third,
                    in1=rop3[rows, :, 1:257],
                    op0=mybir.AluOpType.mult,
                    op1=mybir.AluOpType.add,
                )
                nc.vector.scalar_tensor_tensor(
                    out=rwv[rows, :, 1, :],
                    in0=rop3[rows, :, 2:258],
                    scalar=third,
                    in1=rop3[rows, :, 1:257],
                    op0=mybir.AluOpType.mult,
                    op1=mybir.AluOpType.add,
                )

            def passB_tile(t):
                # output rows 128t..128t+127 read only pooled conv rows
                # < nhi = 2t+3 (hup weights at rows >= nhi are zero; rows
                # below 2t-1 are zero too, but matmul base partitions must
                # be 0/32, so contract from pg — those rw rows are valid
                # from earlier rounds)
                nhi = min(8, 2 * t + 3)
                for q in range(4):
                    outq = wpool.tile([128, 128 * CH_OUT], dt,
                                      tag="outq", bufs=2, name=f"oq{t}{q}")
                    outq3 = outq[:].rearrange("p (x ch) -> p x ch",
                                              ch=CH_OUT)
                    fmq = fmb4[:, t, 128 * q:128 * (q + 1), :]
                    nc.scalar.activation(
                        out=outq3[:, :, 0:16], in_=fmq,
                        func=mybir.ActivationFunctionType.Copy,
                    )
                    for b in range(2):
                        pg = 32 * b
                        lhsT = hup_t[pg:pg + nhi,
                                     128 * t:128 * (t + 1)]
                        for fh in range(2):
                            ps = psall.tile([128, 1024], dt, tag="up",
                                            bufs=2, name=f"psu{t}{q}{b}{fh}")
                            for i in range(2):
                                nc.tensor.matmul(
                                    ps[:, 512 * i:512 * (i + 1)],
                                    lhsT,
                                    rwx[pg:pg + nhi,
                                        8 * fh + 4 * i:8 * fh + 4 * (i + 1),
                                        128 * q:128 * (q + 1)],
                                    start=True, stop=True,
                                )
                            psx = ps[:].rearrange("p (f x) -> p x f", x=128)
                            nc.vector.tensor_sub(
                                outq3[:, :, 16 * (b + 1) + 8 * fh:
                                      16 * (b + 1) + 8 * (fh + 1)],
                                fmq[:, :, 8 * fh:8 * (fh + 1)], psx)
                    nc.sync.dma_start(
                        out=out_d[128 * t:128 * (t + 1),
                                  128 * q:128 * (q + 1), :],
                        in_=outq3,
                    )

            # ---------------- pipelined schedule ----------------
            fmfs0 = load_tile(0)
            nc.sync.dma_start(out=poolw_t[:], in_=poolw_d[:])
            nc.sync.dma_start(out=hup_t[:], in_=hup_d[:])
            nc.sync.dma_start(out=ktaps_t[:], in_=ktaps_d[:])
            nc.sync.dma_start(out=bias_t[:], in_=bias_d[:])
            nc.sync.dma_start(out=nd3[:, :, 0:1], in_=zsrc)
            nc.sync.dma_start(out=nd3[:, :, 257:258], in_=zsrc)
            fmfs1 = load_tile(1)

            cast_tile(0, fmfs0)
            pool_tile(0)
            cast_tile(1, fmfs1)
            pool_tile(1)
            fmfs2 = load_tile(2)

            # conv rows 0-2 (needs pooled rows 0-3), W-up rows 0-2
            for b in range(2):
                conv_unit(b, 0, 2)
                conv_unit(b, 2, 1)
                tail(b, 0, 3)
            wup_round(3)

            cast_tile(2, fmfs2)
            fmfs3 = load_tile(3)

            passB_tile(0)

            pool_tile(2)
            # conv rows 3-4 (needs pooled rows 2-5), W-up rows 3-4
            for b in range(2):
                conv_unit(b, 3, 2)
                tail(b, 3, 5)
            wup_round(5)

            cast_tile(3, fmfs3)

            passB_tile(1)

            pool_tile(3)
            # conv rows 5-7 (needs pooled rows 4-7 + zero halo), W-up 5-7
            for b in range(2):
                conv_unit(b, 5, 2)
                conv_unit(b, 7, 1)
                tail(b, 5, 8)
            wup_round(8)

            passB_tile(2)
            passB_tile(3)
    if compile:
        nc.compile()
    return nc


def _get_program():
    if "nc" not in _cache:
        _cache["nc"] = _build_program()
    return _cache["nc"]


def kernel(feature_map, kernel, bias):
    from concourse.bass_utils import run_bass_kernel_spmd

    feature_map = np.ascontiguousarray(feature_map, dtype=np.float32)
    kernel = np.ascontiguousarray(kernel, dtype=np.float32)
    bias = np.ascontiguousarray(bias, dtype=np.float32)
    B = feature_map.shape[0]
    assert B == 8

    poolw, hup, kt, bias2, _, _ = _host_consts(kernel, bias)
    nc = _get_program()
    in_maps = [
        {
            "feature_map": feature_map[b],
            "poolw": poolw,
            "hup": hup,
            "ktaps": kt,
            "bias2": bias2,
        }
        for b in range(B)
    ]
    res = run_bass_kernel_spmd(nc, in_maps, list(range(B)))
    out = np.stack([res.results[b]["out"] for b in range(B)])
    return out
